# revision 18
# baseline (speedup 1.0000x reference)
"""nn_Net_43860206026847: GRU-like net on 8 trn2 NeuronCores (Bass/Tile).

Strategy (v3)
-------------
Data-parallel over batch (8 rows/core, params replicated), single fused
pass per core:

  * Markov projection folded into the gates (Wp_g = Wg[:, :H] @ Wm); the
    scan keeps only the h-dependent halves A_g = Wg[:, H:].
  * Projections Ug run on-the-fly per 32-step block in bf16 straight into
    SBUF (no DRAM round trip), pre-scaled by WS so they share the fp8
    matmul scale; the descale rides the activation `scale` input.
  * Scan matmuls: fp8(e4m3) weights+operands, plain 128x128 stationary
    tiles (halves the PE weight-load bytes vs fp16).  The scan is
    weight-load bound: 3*H*H weights/step stream through the PE load port.
  * Ug[t] is copied into PSUM before the step's matmuls; all scan matmuls
    accumulate with start=False on top, so sigmoid/tanh read PSUM
    directly - no separate add, and no start-bit zero-region hazards.
  * Elementwise chain split in feature halves; projection groups for the
    next block are interleaved between scan steps as PE filler.
"""

import numpy as np
import ml_dtypes
from contextlib import ExitStack

import concourse.bass as bass
import concourse.tile as tile
from concourse import bacc, mybir
from concourse import bass_utils

B, S, D, H = 64, 512, 768, 1024
NCORES = 8
BL = B // NCORES      # 8 batch rows per core
P = 128
DC = D // P           # 6 contraction chunks over D
HC = H // P           # 8 chunks over H
TB = 32               # scan time-block (projection granularity)
NBLK = S // TB
WS = 1024.0           # scan weight scale (max |A_g|*WS ~ 118 < 240)
INV = 1.0 / WS

F32 = mybir.dt.float32
BF16 = mybir.dt.bfloat16

SCAN_FP8 = False
if SCAN_FP8:
    SDT = mybir.dt.float8e4
    SNP = ml_dtypes.float8_e4m3
else:
    SDT = mybir.dt.float16
    SNP = np.float16


def _host_prep(x, Wm, bm, Wh, bh, Wz, bz, Wr, br, Wi, bi):
    f8 = np.float64
    Wg = [np.asarray(w) for w in (Wz, Wr, Wi)]
    bg = [np.asarray(b) for b in (bz, br, bi)]
    Wp = [np.asarray(W, f8)[:, :H] @ np.asarray(Wm, f8) for W in Wg]
    bp = [np.asarray(b, f8) + np.asarray(W, f8)[:, :H] @ np.asarray(bm, f8)
          for W, b in zip(Wg, bg)]

    WprojT = np.empty((3, DC, P, H), ml_dtypes.bfloat16)
    for g in range(3):
        WprojT[g] = Wp[g].T.astype(np.float32).reshape(DC, P, H)
    bprj = np.stack([(b * WS).astype(np.float32).reshape(HC, P) for b in bp])

    # scan weights: [g, p, kc, jc, m] so (kc, jc) tiles are contiguous
    Ws8 = np.empty((3, P, HC, HC, P), SNP)
    for g in range(3):
        A8 = (np.asarray(Wg[g], np.float32)[:, H:] * WS).astype(SNP)
        Ws8[g] = A8.reshape(HC, P, HC, P).transpose(3, 2, 0, 1)
    Ws8 = np.ascontiguousarray(Ws8.reshape(3, P, HC * HC * P))

    WhT = np.ascontiguousarray(np.asarray(Wh, np.float32).T).reshape(DC, P, H)
    bh_r = np.asarray(bh, np.float32).reshape(HC, P)

    x = np.asarray(x, np.float32)
    in_maps = []
    for c in range(NCORES):
        xc = x[c * BL:(c + 1) * BL]
        xT = np.ascontiguousarray(
            xc.transpose(2, 1, 0).reshape(DC, P, S * BL).astype(ml_dtypes.bfloat16))
        x0T = np.ascontiguousarray(xc[:, 0, :].T.reshape(DC, P, BL))
        in_maps.append({
            "xT": xT, "x0T": x0T, "WprojT": WprojT, "Ws8": Ws8,
            "WhT": WhT, "bprj": bprj, "bh": bh_r,
        })
    return in_maps


def _build_nc():
    nc = bacc.Bacc("TRN2", target_bir_lowering=False, debug=False,
                   num_devices=NCORES)

    xT_in = nc.dram_tensor("xT", [DC, P, S * BL], BF16, kind="ExternalInput").ap()
    x0T_in = nc.dram_tensor("x0T", [DC, P, BL], F32, kind="ExternalInput").ap()
    wproj_in = nc.dram_tensor("WprojT", [3, DC, P, H], BF16, kind="ExternalInput").ap()
    ws8_in = nc.dram_tensor("Ws8", [3, P, HC * HC * P], SDT, kind="ExternalInput").ap()
    wh_in = nc.dram_tensor("WhT", [DC, P, H], F32, kind="ExternalInput").ap()
    bprj_in = nc.dram_tensor("bprj", [3, HC, P], F32, kind="ExternalInput").ap()
    bh_in = nc.dram_tensor("bh", [HC, P], F32, kind="ExternalInput").ap()
    hout = nc.dram_tensor("hout", [HC, P, BL], SDT, kind="ExternalOutput").ap()

    sig = mybir.ActivationFunctionType.Sigmoid
    tanh = mybir.ActivationFunctionType.Tanh
    ident = mybir.ActivationFunctionType.Identity
    copyf = mybir.ActivationFunctionType.Copy
    SUB = mybir.AluOpType.subtract
    MUL = mybir.AluOpType.mult
    ADD = mybir.AluOpType.add

    with tile.TileContext(nc) as tc, ExitStack() as ctx:
        pers = ctx.enter_context(tc.tile_pool(name="pers", bufs=1))

        wproj_sb = pers.tile([P, 3 * DC * H], BF16)
        for g in range(3):
            for kc in range(DC):
                nc.sync.dma_start(
                    wproj_sb[:, (g * DC + kc) * H:(g * DC + kc + 1) * H],
                    wproj_in[g, kc])
        WSZ = HC * HC * P
        ws8_sb = pers.tile([P, 3 * WSZ], SDT)
        for g in range(3):
            nc.sync.dma_start(ws8_sb[:, g * WSZ:(g + 1) * WSZ], ws8_in[g])
        bprj_sb = pers.tile([P, 3 * HC], F32)
        for g in range(3):
            nc.sync.dma_start(bprj_sb[:, g * HC:(g + 1) * HC],
                              bprj_in[g].rearrange("h p -> p h"))
        bh_sb = pers.tile([P, HC], F32)
        nc.sync.dma_start(bh_sb[:], bh_in.rearrange("h p -> p h"))

        def ws8_tile(g, kc, jc):
            base = g * WSZ + (kc * HC + jc) * P
            return ws8_sb[:, base:base + P]

        hpool = ctx.enter_context(tc.tile_pool(name="hpool", bufs=2))
        tmppool = ctx.enter_context(tc.tile_pool(name="tmppool", bufs=2))
        ugzr_pool = ctx.enter_context(tc.tile_pool(name="ugzr", bufs=2))
        ugi_pool = ctx.enter_context(tc.tile_pool(name="ugi", bufs=2))
        xblk_pool = ctx.enter_context(tc.tile_pool(name="xblk", bufs=2))
        psA = ctx.enter_context(tc.tile_pool(name="psA", bufs=4, space="PSUM"))
        pszr = ctx.enter_context(tc.tile_pool(name="pszr", bufs=2, space="PSUM"))
        psi = ctx.enter_context(tc.tile_pool(name="psi", bufs=2, space="PSUM"))

        # ---------------- h0 = x0 @ Wh.T + bh (state lives in fp16) -------
        h16 = hpool.tile([P, HC * BL], SDT, tag="h")
        with ExitStack() as bctx:
            bpool = bctx.enter_context(tc.tile_pool(name="bpool", bufs=1))
            whT_sb = bpool.tile([P, DC * H], F32)
            for kc in range(DC):
                nc.sync.dma_start(whT_sb[:, kc * H:(kc + 1) * H], wh_in[kc])
            x0t = bpool.tile([P, DC * BL], F32)
            for kc in range(DC):
                nc.sync.dma_start(x0t[:, kc * BL:(kc + 1) * BL], x0T_in[kc])
            for fc in range(HC):
                psB = psi.tile([P, HC * BL], F32, tag="i")
                for kc in range(DC):
                    nc.tensor.matmul(
                        psB[:, :BL],
                        whT_sb[:, kc * H + fc * P: kc * H + (fc + 1) * P],
                        x0t[:, kc * BL:(kc + 1) * BL],
                        start=(kc == 0), stop=(kc == DC - 1))
                nc.any.tensor_scalar_add(h16[:, fc * BL:(fc + 1) * BL],
                                         psB[:, :BL], bh_sb[:, fc:fc + 1])

        # ---------------- fused projection + scan ----------------
        def make_proj(blk):
            xt = xblk_pool.tile([P, DC * TB * BL], BF16, tag="xt")
            for kc in range(DC):
                nc.sync.dma_start(
                    xt[:, kc * TB * BL:(kc + 1) * TB * BL],
                    xT_in[kc, :, blk * TB * BL:(blk + 1) * TB * BL])
            ug_zr = ugzr_pool.tile([P, TB * 2 * HC * BL], F32, tag="ugzr")
            ug_i = ugi_pool.tile([P, TB * HC * BL], F32, tag="ugi")
            zr_v = ug_zr[:].rearrange("p (t g j b) -> p t g j b", t=TB, g=2, j=HC)
            i_v = ug_i[:].rearrange("p (t j b) -> p t j b", t=TB, j=HC)

            def group(g, fc):
                def emit():
                    # full-bank tile (2KB/partition) so each projection's
                    # start=True zero-region stays within its own bank
                    pA = psA.tile([P, 512], F32, tag="pA")
                    for kc in range(DC):
                        nc.tensor.matmul(
                            pA[:, :TB * BL],
                            wproj_sb[:, (g * DC + kc) * H + fc * P:
                                     (g * DC + kc) * H + (fc + 1) * P],
                            xt[:, kc * TB * BL:(kc + 1) * TB * BL],
                            start=(kc == 0), stop=(kc == DC - 1))
                    dst = zr_v[:, :, g, fc, :] if g < 2 else i_v[:, :, fc, :]
                    nc.scalar.activation(dst, pA[:, :TB * BL], ident,
                                         bias=bprj_sb[:, g * HC + fc:g * HC + fc + 1],
                                         scale=WS)
                return emit
            groups = [group(g, fc) for g in range(3) for fc in range(HC)]
            return ug_zr, ug_i, groups

        def emit_scan_block(ug_zr, ug_i, filler):
            nonlocal h16
            ugzr_f = ug_zr[:]
            ugi_f = ug_i[:]

            def preload_zr(tau):
                # Ug[tau] into PSUM; the step's matmuls accumulate on top.
                ps_zr = pszr.tile([P, 2 * HC * BL], F32, tag="zr")
                nc.vector.tensor_copy(
                    ps_zr[:], ugzr_f[:, tau * 2 * HC * BL:(tau + 1) * 2 * HC * BL])
                return ps_zr

            def preload_i(tau):
                ps_i = psi.tile([P, HC * BL], F32, tag="i")
                nc.scalar.activation(
                    ps_i[:], ugi_f[:, tau * HC * BL:(tau + 1) * HC * BL], copyf)
                return ps_i

            ps_zr_next = preload_zr(0)
            ps_i_next = preload_i(0)
            for tau in range(TB):
                h_prev = h16
                hv = h_prev[:].rearrange("p (j b) -> p j b", j=HC)

                ps_zr, ps_i = ps_zr_next, ps_i_next
                zrp = ps_zr[:].rearrange("p (g j b) -> p g j b", g=2, j=HC)
                zr_sb = tmppool.tile([P, 2 * HC * BL], SDT, tag="zrsb")
                zrv = zr_sb[:].rearrange("p (g j b) -> p g j b", g=2, j=HC)
                rh8 = tmppool.tile([P, HC * BL], SDT, tag="rh8")
                rh8v = rh8[:].rearrange("p (k b) -> p k b", k=HC)
                piv = ps_i[:].rearrange("p (j b) -> p j b", j=HC)
                hp = tmppool.tile([P, HC * BL], SDT, tag="hp")
                zh = tmppool.tile([P, HC * BL], SDT, tag="zh")
                vv = tmppool.tile([P, HC * BL], SDT, tag="vv")
                w = tmppool.tile([P, HC * BL], SDT, tag="w")
                h_new = hpool.tile([P, HC * BL], SDT, tag="h")

                def zr_mm(g, jc, kcs):
                    for kc in kcs:
                        nc.tensor.matmul(
                            zrp[:, g, jc, :], ws8_tile(g, kc, jc), hv[:, kc],
                            start=False, stop=(kc == HC - 1),
                            skip_group_check=True)

                def i_mm(jc, kcs):
                    for kc in kcs:
                        nc.tensor.matmul(
                            piv[:, jc, :], ws8_tile(2, kc, jc), rh8v[:, kc],
                            start=False, stop=(kc == HC - 1),
                            skip_group_check=True)

                KH0, KH1 = range(0, 4), range(4, HC)
                # z,r for output-half 0 fully first, so sigma-h0 (and with
                # it r*h and the candidate matmuls) unlocks after 64 pairs
                # instead of 96
                for g in range(2):
                    for jc in range(4):
                        zr_mm(g, jc, KH0)
                for g in range(2):
                    for jc in range(4):
                        zr_mm(g, jc, KH1)
                # next step's PSUM preloads, early in both queues
                if tau + 1 < TB:
                    ps_zr_next = preload_zr(tau + 1)
                    ps_i_next = preload_i(tau + 1)
                # per feature half: sigma, r*h, and the early update terms
                # z*h and v = h - z*h (so only 2 DVE ops remain after tanh)
                for jh in range(2):
                    jsl = slice(jh * 4, jh * 4 + 4)
                    csl = slice(jh * 4 * BL, (jh * 4 + 4) * BL)
                    if jh == 1:
                        for g in range(2):
                            for jc in range(4, HC):
                                zr_mm(g, jc, KH0)
                        for g in range(2):
                            for jc in range(4, HC):
                                zr_mm(g, jc, KH1)
                    nc.scalar.activation(zrv[:, :, jsl, :], zrp[:, :, jsl, :],
                                         sig, scale=INV)
                    nc.vector.tensor_tensor(rh8[:, csl], zrv[:, 1, jsl, :],
                                            hv[:, jsl, :], MUL)
                    nc.vector.tensor_tensor(zh[:, csl], zrv[:, 0, jsl, :],
                                            hv[:, jsl, :], MUL)
                    nc.vector.tensor_tensor(vv[:, csl], h_prev[:, csl],
                                            zh[:, csl], SUB)

                # candidate gate
                for jc in range(HC):
                    i_mm(jc, KH0)
                for jc in range(4):
                    i_mm(jc, KH1)
                for jh in range(2):
                    jsl = slice(jh * 4, jh * 4 + 4)
                    csl = slice(jh * 4 * BL, (jh * 4 + 4) * BL)
                    if jh == 1:
                        for jc in range(4, HC):
                            i_mm(jc, KH1)
                    nc.scalar.activation(hp[:, csl], piv[:, jsl, :],
                                         tanh, scale=INV)
                    nc.vector.tensor_tensor(w[:, csl], zrv[:, 0, jsl, :],
                                            hp[:].rearrange("p (j b) -> p j b", j=HC)[:, jsl, :],
                                            MUL)
                    nc.vector.tensor_tensor(h_new[:, csl], vv[:, csl],
                                            w[:, csl], ADD)

                if filler:
                    filler.pop(0)()

                h16 = h_new

        prev = None
        for blk in range(NBLK):
            ug_zr, ug_i, groups = make_proj(blk)
            if prev is None:
                for e in groups:
                    e()
                prev = (ug_zr, ug_i)
            else:
                emit_scan_block(*prev, filler=groups)
                prev = (ug_zr, ug_i)
        emit_scan_block(*prev, filler=[])

        for fc in range(HC):
            nc.sync.dma_start(hout[fc], h16[:, fc * BL:(fc + 1) * BL])

    nc.compile()
    return nc


_NC_CACHE = None


def kernel(**inputs) -> np.ndarray:
    global _NC_CACHE
    in_maps = _host_prep(**{k: np.asarray(v) for k, v in inputs.items()})
    if _NC_CACHE is None:
        _NC_CACHE = _build_nc()
    res = bass_utils.run_bass_kernel_spmd(
        _NC_CACHE, in_maps, core_ids=list(range(NCORES)), trace=False)
    out = np.empty((B, 1, H), np.float32)
    for c, r in enumerate(res.results):
        out[c * BL:(c + 1) * BL, 0, :] = np.asarray(
            r["hout"], np.float32).transpose(2, 0, 1).reshape(BL, H)
    return out


# revision 23
# speedup vs baseline: 1.0133x; 1.0133x over previous
"""nn_Net_43860206026847: GRU-like net on 8 trn2 NeuronCores (Bass/Tile).

Strategy (v3)
-------------
Data-parallel over batch (8 rows/core, params replicated), single fused
pass per core:

  * Markov projection folded into the gates (Wp_g = Wg[:, :H] @ Wm); the
    scan keeps only the h-dependent halves A_g = Wg[:, H:].
  * Projections Ug run on-the-fly per 32-step block in bf16 straight into
    SBUF (no DRAM round trip), pre-scaled by WS so they share the fp8
    matmul scale; the descale rides the activation `scale` input.
  * Scan matmuls: fp8(e4m3) weights+operands, plain 128x128 stationary
    tiles (halves the PE weight-load bytes vs fp16).  The scan is
    weight-load bound: 3*H*H weights/step stream through the PE load port.
  * Ug[t] is copied into PSUM before the step's matmuls; all scan matmuls
    accumulate with start=False on top, so sigmoid/tanh read PSUM
    directly - no separate add, and no start-bit zero-region hazards.
  * Elementwise chain split in feature halves; projection groups for the
    next block are interleaved between scan steps as PE filler.
"""

import numpy as np
import ml_dtypes
from contextlib import ExitStack

import concourse.bass as bass
import concourse.tile as tile
from concourse import bacc, mybir
from concourse import bass_utils

B, S, D, H = 64, 512, 768, 1024
NCORES = 8
BL = B // NCORES      # 8 batch rows per core
P = 128
DC = D // P           # 6 contraction chunks over D
HC = H // P           # 8 chunks over H
TB = 32               # scan time-block (projection granularity)
NBLK = S // TB
WS = 1024.0           # scan weight scale (max |A_g|*WS ~ 118 < 240)
INV = 1.0 / WS

F32 = mybir.dt.float32
BF16 = mybir.dt.bfloat16

SCAN_FP8 = False
if SCAN_FP8:
    SDT = mybir.dt.float8e4
    SNP = ml_dtypes.float8_e4m3
else:
    SDT = mybir.dt.float16
    SNP = np.float16


def _host_prep(x, Wm, bm, Wh, bh, Wz, bz, Wr, br, Wi, bi):
    f8 = np.float64
    Wg = [np.asarray(w) for w in (Wz, Wr, Wi)]
    bg = [np.asarray(b) for b in (bz, br, bi)]
    Wp = [np.asarray(W, f8)[:, :H] @ np.asarray(Wm, f8) for W in Wg]
    bp = [np.asarray(b, f8) + np.asarray(W, f8)[:, :H] @ np.asarray(bm, f8)
          for W, b in zip(Wg, bg)]

    WprojT = np.empty((3, DC, P, H), ml_dtypes.bfloat16)
    for g in range(3):
        WprojT[g] = Wp[g].T.astype(np.float32).reshape(DC, P, H)
    bprj = np.stack([(b * WS).astype(np.float32).reshape(HC, P) for b in bp])

    # scan weights: [g, p, kc, jc, m] so (kc, jc) tiles are contiguous
    Ws8 = np.empty((3, P, HC, HC, P), SNP)
    for g in range(3):
        A8 = (np.asarray(Wg[g], np.float32)[:, H:] * WS).astype(SNP)
        Ws8[g] = A8.reshape(HC, P, HC, P).transpose(3, 2, 0, 1)
    Ws8 = np.ascontiguousarray(Ws8.reshape(3, P, HC * HC * P))

    WhT = np.ascontiguousarray(np.asarray(Wh, np.float32).T).reshape(DC, P, H)
    bh_r = np.asarray(bh, np.float32).reshape(HC, P)

    x = np.asarray(x, np.float32)
    in_maps = []
    for c in range(NCORES):
        xc = x[c * BL:(c + 1) * BL]
        xT = np.ascontiguousarray(
            xc.transpose(2, 1, 0).reshape(DC, P, S * BL).astype(ml_dtypes.bfloat16))
        x0T = np.ascontiguousarray(xc[:, 0, :].T.reshape(DC, P, BL))
        in_maps.append({
            "xT": xT, "x0T": x0T, "WprojT": WprojT, "Ws8": Ws8,
            "WhT": WhT, "bprj": bprj, "bh": bh_r,
        })
    return in_maps


def _build_nc():
    nc = bacc.Bacc("TRN2", target_bir_lowering=False, debug=False,
                   num_devices=NCORES)

    xT_in = nc.dram_tensor("xT", [DC, P, S * BL], BF16, kind="ExternalInput").ap()
    x0T_in = nc.dram_tensor("x0T", [DC, P, BL], F32, kind="ExternalInput").ap()
    wproj_in = nc.dram_tensor("WprojT", [3, DC, P, H], BF16, kind="ExternalInput").ap()
    ws8_in = nc.dram_tensor("Ws8", [3, P, HC * HC * P], SDT, kind="ExternalInput").ap()
    wh_in = nc.dram_tensor("WhT", [DC, P, H], F32, kind="ExternalInput").ap()
    bprj_in = nc.dram_tensor("bprj", [3, HC, P], F32, kind="ExternalInput").ap()
    bh_in = nc.dram_tensor("bh", [HC, P], F32, kind="ExternalInput").ap()
    hout = nc.dram_tensor("hout", [HC, P, BL], SDT, kind="ExternalOutput").ap()

    sig = mybir.ActivationFunctionType.Sigmoid
    tanh = mybir.ActivationFunctionType.Tanh
    ident = mybir.ActivationFunctionType.Identity
    copyf = mybir.ActivationFunctionType.Copy
    SUB = mybir.AluOpType.subtract
    MUL = mybir.AluOpType.mult
    ADD = mybir.AluOpType.add

    with tile.TileContext(nc) as tc, ExitStack() as ctx:
        pers = ctx.enter_context(tc.tile_pool(name="pers", bufs=1))

        wproj_sb = pers.tile([P, 3 * DC * H], BF16)
        for g in range(3):
            for kc in range(DC):
                nc.sync.dma_start(
                    wproj_sb[:, (g * DC + kc) * H:(g * DC + kc + 1) * H],
                    wproj_in[g, kc])
        WSZ = HC * HC * P
        ws8_sb = pers.tile([P, 3 * WSZ], SDT)
        for g in range(3):
            nc.sync.dma_start(ws8_sb[:, g * WSZ:(g + 1) * WSZ], ws8_in[g])
        bprj_sb = pers.tile([P, 3 * HC], F32)
        for g in range(3):
            nc.sync.dma_start(bprj_sb[:, g * HC:(g + 1) * HC],
                              bprj_in[g].rearrange("h p -> p h"))
        bh_sb = pers.tile([P, HC], F32)
        nc.sync.dma_start(bh_sb[:], bh_in.rearrange("h p -> p h"))

        def ws8_tile(g, kc, jc):
            base = g * WSZ + (kc * HC + jc) * P
            return ws8_sb[:, base:base + P]

        hpool = ctx.enter_context(tc.tile_pool(name="hpool", bufs=2))
        tmppool = ctx.enter_context(tc.tile_pool(name="tmppool", bufs=2))
        ugzr_pool = ctx.enter_context(tc.tile_pool(name="ugzr", bufs=2))
        ugi_pool = ctx.enter_context(tc.tile_pool(name="ugi", bufs=2))
        xblk_pool = ctx.enter_context(tc.tile_pool(name="xblk", bufs=2))
        psA = ctx.enter_context(tc.tile_pool(name="psA", bufs=2, space="PSUM"))
        pszr = ctx.enter_context(tc.tile_pool(name="pszr", bufs=2, space="PSUM"))
        psi = ctx.enter_context(tc.tile_pool(name="psi", bufs=2, space="PSUM"))

        # ---------------- h0 = x0 @ Wh.T + bh (state lives in fp16) -------
        # state kept as two half tiles so dependency tracking (which is
        # tile-granular) never serializes half0 consumers on half1 writes
        hA = hpool.tile([P, 4 * BL], SDT, tag="hA")
        hB = hpool.tile([P, 4 * BL], SDT, tag="hB")
        with ExitStack() as bctx:
            bpool = bctx.enter_context(tc.tile_pool(name="bpool", bufs=1))
            whT_sb = bpool.tile([P, DC * H], F32)
            for kc in range(DC):
                nc.sync.dma_start(whT_sb[:, kc * H:(kc + 1) * H], wh_in[kc])
            x0t = bpool.tile([P, DC * BL], F32)
            for kc in range(DC):
                nc.sync.dma_start(x0t[:, kc * BL:(kc + 1) * BL], x0T_in[kc])
            for fc in range(HC):
                psB = psA.tile([P, 512], F32, tag="pA")
                for kc in range(DC):
                    nc.tensor.matmul(
                        psB[:, :BL],
                        whT_sb[:, kc * H + fc * P: kc * H + (fc + 1) * P],
                        x0t[:, kc * BL:(kc + 1) * BL],
                        start=(kc == 0), stop=(kc == DC - 1))
                htile = hA if fc < 4 else hB
                nc.any.tensor_scalar_add(
                    htile[:, (fc % 4) * BL:(fc % 4 + 1) * BL],
                    psB[:, :BL], bh_sb[:, fc:fc + 1])

        # ---------------- fused projection + scan ----------------
        def make_proj(blk):
            xt = xblk_pool.tile([P, DC * TB * BL], BF16, tag="xt")
            for kc in range(DC):
                nc.sync.dma_start(
                    xt[:, kc * TB * BL:(kc + 1) * TB * BL],
                    xT_in[kc, :, blk * TB * BL:(blk + 1) * TB * BL])
            ug_zr = ugzr_pool.tile([P, TB * 2 * HC * BL], F32, tag="ugzr")
            ug_i = ugi_pool.tile([P, TB * HC * BL], F32, tag="ugi")
            # layouts: [t, jh, g, jc', b] / [t, jh, jc', b] so each
            # (tau, half) preload slice is contiguous
            zr_v = ug_zr[:].rearrange("p (t h g j b) -> p t h g j b",
                                      t=TB, h=2, g=2, j=HC // 2)
            i_v = ug_i[:].rearrange("p (t h j b) -> p t h j b",
                                    t=TB, h=2, j=HC // 2)

            def group(g, fc):
                def emit():
                    # full-bank tile (2KB/partition) so each projection's
                    # start=True zero-region stays within its own bank
                    pA = psA.tile([P, 512], F32, tag="pA")
                    for kc in range(DC):
                        nc.tensor.matmul(
                            pA[:, :TB * BL],
                            wproj_sb[:, (g * DC + kc) * H + fc * P:
                                     (g * DC + kc) * H + (fc + 1) * P],
                            xt[:, kc * TB * BL:(kc + 1) * TB * BL],
                            start=(kc == 0), stop=(kc == DC - 1))
                    dst = (zr_v[:, :, fc // 4, g, fc % 4, :] if g < 2
                           else i_v[:, :, fc // 4, fc % 4, :])
                    nc.scalar.activation(dst, pA[:, :TB * BL], ident,
                                         bias=bprj_sb[:, g * HC + fc:g * HC + fc + 1],
                                         scale=WS)
                return emit
            groups = [group(g, fc) for g in range(3) for fc in range(HC)]
            return ug_zr, ug_i, groups

        def emit_scan_block(ug_zr, ug_i, filler):
            nonlocal hA, hB
            HB2 = 4 * BL  # columns per half
            zr_v = ug_zr[:].rearrange("p (t h g j b) -> p t h g j b",
                                      t=TB, h=2, g=2, j=4)
            i_v = ug_i[:].rearrange("p (t h j b) -> p t h j b", t=TB, h=2, j=4)

            def preload_zr(tau):
                pzA = pszr.tile([P, 2 * HB2], F32, tag="zrA", name="pzA")
                pzB = pszr.tile([P, 2 * HB2], F32, tag="zrB", name="pzB")
                nc.vector.tensor_copy(pzA[:], zr_v[:, tau, 0])
                nc.vector.tensor_copy(pzB[:], zr_v[:, tau, 1])
                return pzA, pzB

            def preload_i(tau):
                ps_i = psi.tile([P, 2 * HB2], F32, tag="i", name="ps_i")
                nc.scalar.activation(ps_i[:, :HB2], i_v[:, tau, 0], copyf)
                nc.scalar.activation(ps_i[:, HB2:], i_v[:, tau, 1], copyf)
                return ps_i

            ps_zr_next = preload_zr(0)
            ps_i_next = preload_i(0)
            for tau in range(TB):
                hpA, hpB = hA, hB
                hAv = hpA[:].rearrange("p (j b) -> p j b", j=4)
                hBv = hpB[:].rearrange("p (j b) -> p j b", j=4)

                (pzA, pzB), ps_i = ps_zr_next, ps_i_next
                zrpv = [pzA[:].rearrange("p (g j b) -> p g j b", g=2, j=4),
                        pzB[:].rearrange("p (g j b) -> p g j b", g=2, j=4)]
                zr_sb = [tmppool.tile([P, 2 * HB2], SDT, tag="zrsbA", name="zrsbA"),
                         tmppool.tile([P, 2 * HB2], SDT, tag="zrsbB", name="zrsbB")]
                zrv = [z[:].rearrange("p (g j b) -> p g j b", g=2, j=4)
                       for z in zr_sb]
                rh = [tmppool.tile([P, HB2], SDT, tag="rhA", name="rhA"),
                      tmppool.tile([P, HB2], SDT, tag="rhB", name="rhB")]
                rhv = [r[:].rearrange("p (j b) -> p j b", j=4) for r in rh]
                piv = [ps_i[:, :HB2].rearrange("p (j b) -> p j b", j=4),
                       ps_i[:, HB2:].rearrange("p (j b) -> p j b", j=4)]
                hp = [tmppool.tile([P, HB2], SDT, tag="hpA", name="hpA"),
                      tmppool.tile([P, HB2], SDT, tag="hpB", name="hpB")]
                zh = [tmppool.tile([P, HB2], SDT, tag="zhA", name="zhA"),
                      tmppool.tile([P, HB2], SDT, tag="zhB", name="zhB")]
                vv = [tmppool.tile([P, HB2], SDT, tag="vvA", name="vvA"),
                      tmppool.tile([P, HB2], SDT, tag="vvB", name="vvB")]
                w = [tmppool.tile([P, HB2], SDT, tag="wA", name="wA"),
                     tmppool.tile([P, HB2], SDT, tag="wB", name="wB")]
                h_new = [hpool.tile([P, HB2], SDT, tag="hA", name="hnA"),
                         hpool.tile([P, HB2], SDT, tag="hB", name="hnB")]

                def zr_mm(jh, g, jc, kcs):
                    for kc in kcs:
                        mov = hAv[:, kc, :] if kc < 4 else hBv[:, kc - 4, :]
                        nc.tensor.matmul(
                            zrpv[jh][:, g, jc, :], ws8_tile(g, kc, jh * 4 + jc),
                            mov, start=False, stop=(kc == HC - 1),
                            skip_group_check=True)

                def i_mm(jh, jc, kcs):
                    for kc in kcs:
                        mov = rhv[0][:, kc, :] if kc < 4 else rhv[1][:, kc - 4, :]
                        nc.tensor.matmul(
                            piv[jh][:, jc, :], ws8_tile(2, kc, jh * 4 + jc),
                            mov, start=False, stop=(kc == HC - 1),
                            skip_group_check=True)

                KH0, KH1 = range(0, 4), range(4, HC)

                def zr_elem(jh):
                    hv = hAv if jh == 0 else hBv
                    hprev = hpA if jh == 0 else hpB
                    nc.scalar.activation(zrv[jh][:, :, :, :],
                                         zrpv[jh][:, :, :, :], sig, scale=INV)
                    nc.vector.tensor_tensor(rh[jh][:], zrv[jh][:, 1], hv[:],
                                            MUL)
                    nc.vector.tensor_tensor(zh[jh][:], zrv[jh][:, 0], hv[:],
                                            MUL)
                    nc.vector.tensor_tensor(vv[jh][:], hprev[:], zh[jh][:],
                                            SUB)

                # z,r half A fully, sigma unlocks while half B runs on PE
                for g in range(2):
                    for jc in range(4):
                        zr_mm(0, g, jc, KH0)
                for g in range(2):
                    for jc in range(4):
                        zr_mm(0, g, jc, KH1)
                if tau + 1 < TB:
                    ps_zr_next = preload_zr(tau + 1)
                    ps_i_next = preload_i(tau + 1)
                zr_elem(0)
                for g in range(2):
                    for jc in range(4):
                        zr_mm(1, g, jc, KH0)
                for g in range(2):
                    for jc in range(4):
                        zr_mm(1, g, jc, KH1)
                zr_elem(1)

                # candidate gate: all matmuls first (single ps_i tile;
                # an interleaved tanh read would falsely serialize groups)
                for jh in range(2):
                    for jc in range(4):
                        i_mm(jh, jc, KH0)
                for jh in range(2):
                    for jc in range(4):
                        i_mm(jh, jc, KH1)
                for jh in range(2):
                    nc.scalar.activation(hp[jh][:], piv[jh][:, :, :],
                                         tanh, scale=INV)
                    nc.vector.tensor_tensor(w[jh][:], zrv[jh][:, 0],
                                            hp[jh][:], MUL)
                    nc.vector.tensor_tensor(h_new[jh][:], vv[jh][:],
                                            w[jh][:], ADD)

                if filler:
                    filler.pop(0)()

                hA, hB = h_new

        prev = None
        for blk in range(NBLK):
            ug_zr, ug_i, groups = make_proj(blk)
            if prev is None:
                for e in groups:
                    e()
                prev = (ug_zr, ug_i)
            else:
                emit_scan_block(*prev, filler=groups)
                prev = (ug_zr, ug_i)
        emit_scan_block(*prev, filler=[])

        for fc in range(HC):
            ht = hA if fc < 4 else hB
            nc.sync.dma_start(hout[fc], ht[:, (fc % 4) * BL:(fc % 4 + 1) * BL])

    nc.compile()
    return nc


_NC_CACHE = None


def kernel(**inputs) -> np.ndarray:
    global _NC_CACHE
    in_maps = _host_prep(**{k: np.asarray(v) for k, v in inputs.items()})
    if _NC_CACHE is None:
        _NC_CACHE = _build_nc()
    res = bass_utils.run_bass_kernel_spmd(
        _NC_CACHE, in_maps, core_ids=list(range(NCORES)), trace=False)
    out = np.empty((B, 1, H), np.float32)
    for c, r in enumerate(res.results):
        out[c * BL:(c + 1) * BL, 0, :] = np.asarray(
            r["hout"], np.float32).transpose(2, 0, 1).reshape(BL, H)
    return out
